# revision 6
# baseline (speedup 1.0000x reference)
# Trainium2 Bass kernel for an 8-layer dense transformer (B=4, T=1024,
# V=E=1024, H=16, M=4096), 8-way SPMD across one chip.
#
# Sharding: data-parallel over (batch x 2 interleaved token chunks) ->
# 8 shards of 512 tokens.  Even cores own logical 256-token chunks (0,3)
# of their batch, odd cores own (1,2), which balances causal-attention
# work.  Per layer, each core projects Q/K/V for its own tokens, local
# K/V are exchanged through a single 8-rank AllGather, and everything
# else (LN, QKV, attention, MLP) runs locally with replicated weights.
#
# Layouts: the residual stream is feature-major ([E on partitions,
# tokens on free dim]) so every GEMM consumes the natural row-major
# weight layout with zero transposes.  Scores are key-major so the PV
# matmul is native; the softmax denominator comes out of the same PV
# matmul via a 65th all-ones column appended to V; softmax skips the
# max-subtraction (logits are O(1) here).  Causal masking is a per-core
# 0/1 multiplicative mask tensor so the compiled program is identical
# on all 8 cores; cross-partition LN stats/broadcasts go through tiny
# ones-matmuls on the tensor engine.
#
# All matmuls run in float32r (fp32 storage, ~11 mantissa bits, full PE
# rate at N>=256).  Weights are pre-rounded to fp32r on the host (RNE
# on the low 12 bits) and DMA'd directly into fp32r SBUF tiles.

import os
import sys

for _p in ("/opt/trn_rl_repo", "/root/.axon_site/_ro/trn_rl_repo"):
    if _p not in sys.path and os.path.isdir(_p):
        sys.path.insert(0, _p)

import numpy as np

import concourse.bass as bass
import concourse.mybir as mybir
import concourse.tile as tile
from concourse import bacc
from concourse.bass_utils import run_bass_kernel_spmd

B, T, V, E, H, M, L, C = 4, 1024, 1024, 1024, 16, 4096, 8, 64
NC = 8          # cores
S = 512         # tokens per core
CH = 256        # chunk size
EPS = 1e-5
SCALE = 1.0 / 8.0   # 1/sqrt(C)

F32 = mybir.dt.float32
F32R = mybir.dt.float32r
AOT = mybir.AluOpType
AFT = mybir.ActivationFunctionType

N_LAYERS = int(os.environ.get("KERNEL_LAYERS", str(L)))
DEBUG_X = bool(int(os.environ.get("KERNEL_DEBUG_X", "0")))

# Gathered key-chunk slot -> logical chunk (AG concat: even core's
# chunks (0,3) then odd core's (1,2)).
GORDER = [0, 3, 1, 2]
# Core parity -> logical chunks of its two local q-slots.
QCH = [(0, 3), (1, 2)]

BLOB = S * 2 * E          # 1048576 elems per rank blob (K + V)
VOFF = 8 * 2 * C * S      # 524288; V region: [512 rows, 1024] rows=local token

# P-tile column base for each (g, kt) score block.
PCOL = {(0, 0): 0, (0, 1): 512, (2, 0): 1024, (2, 1): 1536,
        (1, 0): 2048, (1, 1): 2304, (3, 0): 2560, (3, 1): 2816}
BLK512 = [(0, 0), (0, 1), (2, 0), (2, 1)]   # N=512 (both q slots)
BLK256 = [(1, 0), (1, 1), (3, 0), (3, 1)]   # N=256 (q slot 1 only)
# PV accumulation order: first/last must be full-width (N=512) blocks.
PV_ORDER = [(0, 0), (1, 0), (1, 1), (3, 0), (3, 1), (0, 1), (2, 0), (2, 1)]


def rne12(x: np.ndarray) -> np.ndarray:
    """Round fp32 to float32r (round-to-nearest-even, drop low 12 bits)."""
    x = np.ascontiguousarray(x, dtype=np.float32)
    u = x.view(np.uint32).astype(np.uint64)
    lsb = (u >> np.uint64(12)) & np.uint64(1)
    u2 = (u + np.uint64(0x7FF) + lsb) >> np.uint64(12) << np.uint64(12)
    return u2.astype(np.uint32).view(np.float32).reshape(x.shape)


def build_program():
    nc = bacc.Bacc("TRN2", target_bir_lowering=False, debug=False, num_devices=NC)

    # ---- DRAM I/O ----
    d_toksT = nc.dram_tensor("toksT", [V, S], F32R, kind="ExternalInput")
    d_posT = nc.dram_tensor("posT", [E, S], F32, kind="ExternalInput")
    d_maska = nc.dram_tensor("maska", [128, 4, CH], F32, kind="ExternalInput")
    d_maskb = nc.dram_tensor("maskb", [128, 4 * CH], F32, kind="ExternalInput")
    d_pboff = nc.dram_tensor("pboff", [1, 1], mybir.dt.uint32, kind="ExternalInput")
    d_wqk = nc.dram_tensor("wqk", [L, E, H, 2 * C], F32R, kind="ExternalInput")
    d_wv = nc.dram_tensor("wv", [L, E, H * C], F32R, kind="ExternalInput")
    d_w1 = nc.dram_tensor("w1", [L, E, M], F32R, kind="ExternalInput")
    d_w2 = nc.dram_tensor("w2", [L, M, E], F32R, kind="ExternalInput")
    d_b1 = nc.dram_tensor("b1", [L, M], F32, kind="ExternalInput")
    d_b2 = nc.dram_tensor("b2", [L, E], F32R, kind="ExternalInput")
    d_ln1g = nc.dram_tensor("ln1g", [L, E], F32, kind="ExternalInput")
    d_ln1b = nc.dram_tensor("ln1b", [L, E], F32, kind="ExternalInput")
    d_ln2g = nc.dram_tensor("ln2g", [L, E], F32, kind="ExternalInput")
    d_ln2b = nc.dram_tensor("ln2b", [L, E], F32, kind="ExternalInput")
    d_lnfg = nc.dram_tensor("lnfg", [E], F32, kind="ExternalInput")
    d_lnfb = nc.dram_tensor("lnfb", [E], F32, kind="ExternalInput")
    d_tokw = nc.dram_tensor("tokw", [V, E], F32R, kind="ExternalInput")
    d_uw = nc.dram_tensor("uw", [E, V], F32R, kind="ExternalInput")
    d_ub = nc.dram_tensor("ub", [V], F32R, kind="ExternalInput")
    d_out = nc.dram_tensor("logits", [V, S], F32, kind="ExternalOutput")

    with tile.TileContext(nc) as tc:
        from contextlib import ExitStack
        top = ExitStack()
        pers = top.enter_context(tc.tile_pool(name="pers", bufs=1))
        small = top.enter_context(tc.tile_pool(name="small", bufs=1))
        stage = top.enter_context(tc.tile_pool(name="stage", bufs=2))
        bigpool = top.enter_context(tc.tile_pool(name="bigpool", bufs=1))
        dram = top.enter_context(tc.tile_pool(name="dram", bufs=2, space="DRAM"))
        dram_sh = top.enter_context(tc.tile_pool(name="dram_sh", bufs=2, space="DRAM"))

        # ---- persistent SBUF state ----
        x = pers.tile([128, 8, S], F32)        # residual (feature-major)
        inner = pers.tile([128, 8, S], F32)    # x + attn_out (+ mlp out)
        h = pers.tile([128, 8, S], F32R)       # LN output / rounded x
        qsb = pers.tile([128, 8, S], F32R)     # Q (head-pair-major)
        maska = pers.tile([128, 4, CH], F32)
        maskb = pers.tile([128, 4 * CH], F32)
        ones_col = pers.tile([128, 1], F32R)   # stats lhsT
        ones_row = pers.tile([1, 128], F32R)   # broadcast lhsT
        neg_row = pers.tile([1, 128], F32R)
        ones_s = pers.tile([1, S], F32R)       # bias outer-product rhs
        g1 = pers.tile([128, 8], F32)
        bb1 = pers.tile([128, 8], F32)
        g2 = pers.tile([128, 8], F32)
        bb2 = pers.tile([128, 8], F32)
        gf = pers.tile([128, 8], F32)
        bf = pers.tile([128, 8], F32)
        b1sb = pers.tile([128, 32], F32)
        b2r = pers.tile([1, E], F32R)
        ubr = pers.tile([1, V], F32R)
        lnst = small.tile([1, 4 * S], F32)     # mu | msq | var | rvar
        lnst_r = small.tile([1, 2 * S], F32R)  # rstd | nmu

        nc.vector.memset(ones_col[:].bitcast(F32), 1.0)
        nc.vector.memset(ones_row[:].bitcast(F32), 1.0)
        nc.vector.memset(neg_row[:].bitcast(F32), -1.0)
        nc.vector.memset(ones_s[:].bitcast(F32), 1.0)
        nc.sync.dma_start(maska[:], d_maska[:])
        nc.sync.dma_start(maskb[:], d_maskb[:])
        nc.sync.dma_start(gf[:], d_lnfg[:].rearrange("(o p) -> p o", p=128))
        nc.sync.dma_start(bf[:], d_lnfb[:].rearrange("(o p) -> p o", p=128))
        nc.sync.dma_start(ubr[:], d_ub[None, :])

        # pair-base register: element offset of my pair's even-rank blob
        # in the AllGather output.
        pbreg_t = nc.sync.alloc_register("pbreg")
        nc.sync.reg_load(pbreg_t, d_pboff[0:1, 0:1])
        pb = nc.sync.snap(pbreg_t, donate=True, min_val=0, max_val=6 * BLOB)

        # ---------------- helpers ----------------
        def layer_norm(src, g_ap, b_ap):
            """src: [128,8,S] F32 sbuf -> h (F32R).  h doubles as the
            rounded copy of src that feeds the stats matmuls."""
            mu = lnst[:, 0:S]
            msq = lnst[:, S:2 * S]
            var = lnst[:, 2 * S:3 * S]
            rvar = lnst[:, 3 * S:4 * S]
            rstd = lnst_r[:, 0:S]
            nmu = lnst_r[:, S:2 * S]
            with tc.tile_pool(name="ps_ln", bufs=2, space="PSUM") as ps_ln:
                for t in range(8):
                    nc.vector.tensor_copy(h[:, t, :], src[:, t, :])
                st_sum = ps_ln.tile([1, S], F32, tag="stat")
                st_sq = ps_ln.tile([1, S], F32, tag="stat")
                for t in range(8):
                    nc.tensor.matmul(st_sum[:], ones_col[:], h[:, t, :],
                                     start=(t == 0), stop=(t == 7))
                for t in range(8):
                    sqt = stage.tile([128, S], F32R, tag="sqt", bufs=2)
                    nc.vector.tensor_tensor(sqt[:], src[:, t, :], src[:, t, :],
                                            AOT.mult)
                    nc.tensor.matmul(st_sq[:], ones_col[:], sqt[:],
                                     start=(t == 0), stop=(t == 7))
                nc.vector.tensor_scalar_mul(mu, st_sum[:], 1.0 / E)
                nc.vector.tensor_scalar_mul(msq, st_sq[:], 1.0 / E)
                nc.vector.tensor_tensor(var, mu, mu, AOT.mult)
                nc.vector.tensor_tensor(var, msq, var, AOT.subtract)
                nc.vector.tensor_scalar_add(var, var, EPS)
                nc.vector.reciprocal(rvar, var)
                with nc.allow_low_precision(reason="fp32r rstd for matmul bcast"):
                    nc.scalar.sqrt(rstd, rvar)
                nc.vector.tensor_tensor(nmu, mu, rstd, AOT.mult)
                bc_rstd = ps_ln.tile([128, S], F32, tag="bc")
                bc_nmu = ps_ln.tile([128, S], F32, tag="bc")
                nc.tensor.matmul(bc_rstd[:], ones_row[:], rstd,
                                 start=True, stop=True)
                nc.tensor.matmul(bc_nmu[:], neg_row[:], nmu,
                                 start=True, stop=True)
                for t in range(8):
                    tmp = stage.tile([128, S], F32, tag="lntmp", bufs=2)
                    nc.vector.tensor_tensor(tmp[:], src[:, t, :], bc_rstd[:],
                                            AOT.mult)
                    nc.vector.tensor_tensor(tmp[:], tmp[:], bc_nmu[:], AOT.add)
                    nc.vector.tensor_scalar(h[:, t, :], tmp[:],
                                            g_ap[:, t:t + 1], b_ap[:, t:t + 1],
                                            AOT.mult, AOT.add)

        # ---------------- embedding ----------------
        with tc.tile_pool(name="emb", bufs=1) as emb, \
             tc.tile_pool(name="wemb", bufs=2) as wemb, \
             tc.tile_pool(name="ps_emb", bufs=4, space="PSUM") as ps_emb:
            toksr = emb.tile([128, 8, S], F32R)
            for v in range(8):
                nc.sync.dma_start(toksr[:, v, :], d_toksT[v * 128:(v + 1) * 128, :])
            for eg in range(4):
                psl = [ps_emb.tile([128, S], F32, tag="embps", name=f"embps{_i}") for _i in range(2)]
                for vh in range(4):
                    tw = wemb.tile([128, 2, 256], F32R, tag="twt")
                    for v2 in range(2):
                        nc.sync.dma_start(
                            tw[:, v2, :],
                            d_tokw[(vh * 2 + v2) * 128:(vh * 2 + v2 + 1) * 128,
                                   eg * 256:(eg + 1) * 256])
                    for v2 in range(2):
                        v = vh * 2 + v2
                        for eo2 in range(2):
                            eo = eg * 2 + eo2
                            nc.tensor.matmul(
                                psl[eo2][:],
                                tw[:, v2, eo2 * 128:(eo2 + 1) * 128],
                                toksr[:, v, :],
                                start=(v == 0), stop=(v == 7))
                for eo2 in range(2):
                    nc.scalar.copy(x[:, eg * 2 + eo2, :], psl[eo2][:])
        with tc.tile_pool(name="embp", bufs=1) as embp:
            possb = embp.tile([128, 8, S], F32)
            for e in range(8):
                nc.sync.dma_start(possb[:, e, :], d_posT[e * 128:(e + 1) * 128, :])
                nc.vector.tensor_tensor(x[:, e, :], x[:, e, :], possb[:, e, :],
                                        AOT.add)

        # ---------------- layers ----------------
        for l in range(N_LAYERS):
            nc.sync.dma_start(g1[:], d_ln1g[l].rearrange("(o p) -> p o", p=128))
            nc.sync.dma_start(bb1[:], d_ln1b[l].rearrange("(o p) -> p o", p=128))
            nc.sync.dma_start(g2[:], d_ln2g[l].rearrange("(o p) -> p o", p=128))
            nc.sync.dma_start(bb2[:], d_ln2b[l].rearrange("(o p) -> p o", p=128))
            nc.sync.dma_start(b1sb[:], d_b1[l].rearrange("(o p) -> p o", p=128))
            nc.sync.dma_start(b2r[:], d_b2[l][None, :])

            ag_in = dram.tile([BLOB], F32R, tag="agin")
            ag_out = dram_sh.tile([NC * BLOB], F32R, tag="agout",
                                  addr_space="Shared")
            agi_k = ag_in[:].rearrange("(r c) -> r c", c=S)   # [1024, 512]

            # ---- LN1 ----
            layer_norm(x, g1, bb1)

            # ---- QKV projection (Q,K feature-major; K straight to DRAM) ----
            with tc.tile_pool(name="wqkv", bufs=2) as wqkv, \
                 tc.tile_pool(name="ps_qkv", bufs=8, space="PSUM") as ps_qkv:
                for hg in range(2):
                    qk_ps = [ps_qkv.tile([128, S], F32, tag="qkps", bufs=8,
                                         name=f"qkps{_i}") for _i in range(8)]
                    for e in range(8):
                        wq = wqkv.tile([128, 8, 128], F32R, tag="wqk", bufs=2)
                        nc.sync.dma_start(
                            wq[:], d_wqk[l, e * 128:(e + 1) * 128,
                                         hg * 8:(hg + 1) * 8, :])
                        for hh in range(8):
                            nc.tensor.matmul(qk_ps[hh][:], wq[:, hh, :],
                                             h[:, e, :],
                                             start=(e == 0), stop=(e == 7))
                    for hh in range(8):
                        ha = hg * 8 + hh
                        hp, par = ha // 2, ha % 2
                        nc.scalar.copy(qsb[par * 64:par * 64 + 64, hp, :],
                                       qk_ps[hh][0:64, :])
                        kst = stage.tile([64, S], F32R, tag="kst", bufs=3)
                        nc.scalar.copy(kst[:], qk_ps[hh][64:128, :])
                        nc.sync.dma_start(
                            agi_k[hp * 128 + par * 64: hp * 128 + par * 64 + 64,
                                  :],
                            kst[:])
                # ---- V projection (token-major) ----
                v_ps = [[ps_qkv.tile([128, 512], F32, tag="qkps", bufs=8,
                                     name=f"vps{_i}_{_j}")
                         for _j in range(2)] for _i in range(4)]
                for e in range(8):
                    wv = wqkv.tile([128, 1024], F32R, tag="wv", bufs=2)
                    nc.sync.dma_start(wv[:], d_wv[l, e * 128:(e + 1) * 128, :])
                    for tt in range(4):
                        for hf in range(2):
                            nc.tensor.matmul(
                                v_ps[tt][hf][:],
                                h[:, e, tt * 128:(tt + 1) * 128],
                                wv[:, hf * 512:(hf + 1) * 512],
                                start=(e == 0), stop=(e == 7))
                for tt in range(4):
                    vst = stage.tile([128, 1024], F32R, tag="vst", bufs=2)
                    nc.scalar.copy(vst[:, 0:512], v_ps[tt][0][:])
                    nc.scalar.copy(vst[:, 512:1024], v_ps[tt][1][:])
                    nc.sync.dma_start(
                        ag_in[VOFF + tt * 128 * 1024: VOFF + (tt + 1) * 128 * 1024]
                        .rearrange("(r c) -> r c", c=1024),
                        vst[:])

            # ---- KV exchange ----
            nc.gpsimd.collective_compute(
                "AllGather", AOT.bypass,
                replica_groups=[list(range(NC))],
                ins=[ag_in.opt()], outs=[ag_out.opt()])

            # gathered V (token-major, with interleaved ones column)
            vgr = bigpool.tile([128, 8, H, C + 1], F32R, tag="vgrm", bufs=1)
            for kt in range(8):
                blob = pb if kt < 4 else pb + BLOB
                src = ag_out[bass.ds(blob + VOFF + (kt % 4) * 128 * 1024,
                                     128 * 1024)].rearrange(
                    "(r hh cc) -> r hh cc", hh=H, cc=C)
                nc.sync.dma_start(vgr[:, kt, :, 0:C], src)
            nc.vector.memset(vgr[:, :, :, C:C + 1].bitcast(F32), 1.0)

            # ---- attention ----
            with tc.tile_pool(name="attnp", bufs=1) as attnp, \
                 tc.tile_pool(name="ps_s", bufs=4, space="PSUM") as ps_s, \
                 tc.tile_pool(name="ps_y", bufs=2, space="PSUM") as ps_y, \
                 tc.tile_pool(name="ps_by", bufs=2, space="PSUM") as ps_by:
                for hp in range(8):
                    kr = attnp.tile([128, 1024], F32R, tag="kr", bufs=2)
                    nc.sync.dma_start(
                        kr[:, 0:512],
                        ag_out[bass.ds(pb + hp * 128 * S, 128 * S)]
                        .rearrange("(r c) -> r c", c=S))
                    nc.sync.dma_start(
                        kr[:, 512:1024],
                        ag_out[bass.ds(pb + BLOB + hp * 128 * S, 128 * S)]
                        .rearrange("(r c) -> r c", c=S))
                    for head in range(2):
                        ha = hp * 2 + head
                        qb = head * 64
                        P = attnp.tile([128, 3072], F32R, tag="P", bufs=1)
                        for (g, kt) in BLK512:
                            ktg = g * 2 + kt
                            sps = ps_s.tile([128, 512], F32, tag="S", bufs=4)
                            nc.tensor.matmul(
                                sps[:],
                                kr[qb:qb + 64, ktg * 128:(ktg + 1) * 128],
                                qsb[qb:qb + 64, hp, :],
                                start=True, stop=True)
                            pc = PCOL[(g, kt)]
                            nc.scalar.activation(P[:, pc:pc + 512], sps[:],
                                                 AFT.Exp, scale=SCALE)
                        for (g, kt) in BLK256:
                            ktg = g * 2 + kt
                            sps = ps_s.tile([128, 256], F32, tag="S", bufs=4)
                            nc.tensor.matmul(
                                sps[:],
                                kr[qb:qb + 64, ktg * 128:(ktg + 1) * 128],
                                qsb[qb:qb + 64, hp, 256:512],
                                start=True, stop=True)
                            pc = PCOL[(g, kt)]
                            nc.scalar.activation(P[:, pc:pc + 256], sps[:],
                                                 AFT.Exp, scale=SCALE)
                        # causal mask (0/1 multiplicative)
                        ap1 = P[:, 0:2048].rearrange(
                            "p (b q) -> p b q", q=512)[:, :, 0:CH]
                        nc.vector.tensor_tensor(ap1, ap1, maska[:], AOT.mult)
                        ap2 = P[:, 2048:3072]
                        nc.vector.tensor_tensor(ap2, ap2, maskb[:], AOT.mult)
                        # PV (+ denominator via ones column)
                        yps = ps_y.tile([128, 512], F32, tag="y", bufs=2)
                        for i, (g, kt) in enumerate(PV_ORDER):
                            ktg = g * 2 + kt
                            pc = PCOL[(g, kt)]
                            n = 512 if (g, kt) in BLK512 else 256
                            qoff = 0 if n == 512 else 256
                            nc.tensor.matmul(
                                yps[0:65, qoff:qoff + n],
                                vgr[:, ktg, ha, :],
                                P[:, pc:pc + n],
                                start=(i == 0), stop=(i == len(PV_ORDER) - 1))
                        rd = small.tile([1, S], F32R, tag="rd", bufs=2)
                        with nc.allow_low_precision(reason="fp32r 1/d for bcast"):
                            nc.vector.reciprocal(rd[:], yps[64:65, :])
                        bcd = ps_by.tile([64, S], F32, tag="bcd", bufs=2)
                        nc.tensor.matmul(bcd[:], ones_row[:, 0:64], rd[:],
                                         start=True, stop=True)
                        et, ebase = ha // 2, (ha % 2) * 64
                        ysb = stage.tile([128, S], F32, tag="ysb", bufs=2)
                        ysl = ysb[ebase:ebase + 64, :]
                        nc.scalar.copy(ysl, yps[0:64, :])
                        nc.vector.tensor_tensor(ysl, ysl, bcd[:], AOT.mult)
                        # inner = x + y  (y slice: feature rows of head ha)
                        nc.vector.tensor_tensor(
                            inner[ebase:ebase + 64, et, :],
                            x[ebase:ebase + 64, et, :], ysl, AOT.add)

            # ---- LN2 + MLP (mlp out accumulates into `inner`) ----
            layer_norm(inner, g2, bb2)
            with tc.tile_pool(name="wmlp", bufs=1) as wmlp, \
                 tc.tile_pool(name="ps_m", bufs=5, space="PSUM") as ps_m, \
                 tc.tile_pool(name="ps_o", bufs=3, space="PSUM") as ps_o:
                for half in range(2):
                    m_sb = bigpool.tile([128, 16, 512], F32R, tag="vgrm", bufs=1)
                    for og4 in range(4):
                        og = half * 4 + og4
                        mps_l = [ps_m.tile([128, S], F32, tag="mps", bufs=5,
                                           name=f"mps{_i}") for _i in range(4)]
                        for eh in range(2):
                            w1h = wmlp.tile([128, 4, 512], F32R, tag="w1h",
                                            bufs=2)
                            for e4 in range(4):
                                e = eh * 4 + e4
                                nc.sync.dma_start(
                                    w1h[:, e4, :],
                                    d_w1[l, e * 128:(e + 1) * 128,
                                         og * 512:(og + 1) * 512])
                            for ob in range(4):
                                for e4 in range(4):
                                    e = eh * 4 + e4
                                    nc.tensor.matmul(
                                        mps_l[ob][:],
                                        w1h[:, e4, ob * 128:(ob + 1) * 128],
                                        h[:, e, :],
                                        start=(e == 0), stop=(e == 7))
                        for ob in range(4):
                            mtl = og4 * 4 + ob
                            mt_abs = half * 16 + mtl
                            nc.scalar.activation(
                                m_sb[:, mtl, :], mps_l[ob][:], AFT.Relu,
                                bias=b1sb[:, mt_abs:mt_abs + 1], scale=1.0)
                    for eop in range(4):
                        ops = [ps_o.tile([128, S], F32, tag="ops", bufs=3,
                                         name=f"ops{_i}") for _i in range(2)]
                        if half == 0:
                            for eo2 in range(2):
                                eo = eop * 2 + eo2
                                nc.tensor.matmul(
                                    ops[eo2][:],
                                    b2r[:, eo * 128:(eo + 1) * 128],
                                    ones_s[:], start=True, stop=False)
                        for mt in range(16):
                            mt_abs = half * 16 + mt
                            w2t = wmlp.tile([128, 256], F32R, tag="w2", bufs=3)
                            nc.sync.dma_start(
                                w2t[:], d_w2[l, mt_abs * 128:(mt_abs + 1) * 128,
                                             eop * 256:(eop + 1) * 256])
                            for eo2 in range(2):
                                nc.tensor.matmul(
                                    ops[eo2][:],
                                    w2t[:, eo2 * 128:(eo2 + 1) * 128],
                                    m_sb[:, mt, :],
                                    start=(half == 1 and mt == 0),
                                    stop=(mt == 15))
                        for eo2 in range(2):
                            eo = eop * 2 + eo2
                            nc.vector.tensor_tensor(inner[:, eo, :],
                                                    inner[:, eo, :],
                                                    ops[eo2][:], AOT.add)
            # x = x + inner  (inner now holds x + attn + mlp + b2)
            for e in range(8):
                nc.vector.tensor_tensor(x[:, e, :], x[:, e, :], inner[:, e, :],
                                        AOT.add)

        # ---------------- final LN + unembed ----------------
        if DEBUG_X:
            for e in range(8):
                xs = stage.tile([128, S], F32, tag="lntmp", bufs=2)
                nc.vector.tensor_copy(xs[:], x[:, e, :])
                nc.sync.dma_start(d_out[e * 128:(e + 1) * 128, :], xs[:])
        else:
            layer_norm(x, gf, bf)
            with tc.tile_pool(name="wu", bufs=2) as wu, \
                 tc.tile_pool(name="ps_u", bufs=5, space="PSUM") as ps_u:
                for vg in range(2):
                    upl = [ps_u.tile([128, S], F32, tag="ups", bufs=5,
                                     name=f"ups{_i}") for _i in range(4)]
                    for vo4 in range(4):
                        vo = vg * 4 + vo4
                        nc.tensor.matmul(upl[vo4][:],
                                         ubr[:, vo * 128:(vo + 1) * 128],
                                         ones_s[:], start=True, stop=False)
                    for eh in range(2):
                        uwh = wu.tile([128, 4, 512], F32R, tag="uwh", bufs=2)
                        for e4 in range(4):
                            e = eh * 4 + e4
                            nc.sync.dma_start(
                                uwh[:, e4, :],
                                d_uw[e * 128:(e + 1) * 128,
                                     vg * 512:(vg + 1) * 512])
                        for vo4 in range(4):
                            for e4 in range(4):
                                e = eh * 4 + e4
                                nc.tensor.matmul(
                                    upl[vo4][:],
                                    uwh[:, e4, vo4 * 128:(vo4 + 1) * 128],
                                    h[:, e, :], start=False, stop=(e == 7))
                    for vo4 in range(4):
                        vo = vg * 4 + vo4
                        lst = stage.tile([128, S], F32, tag="lntmp", bufs=2)
                        nc.scalar.copy(lst[:], upl[vo4][:])
                        nc.sync.dma_start(d_out[vo * 128:(vo + 1) * 128, :],
                                          lst[:])

        top.close()

    nc.compile()
    return nc


def core_token_idx(c: int) -> np.ndarray:
    p = c % 2
    c0, c1 = QCH[p]
    return np.concatenate([np.arange(c0 * CH, (c0 + 1) * CH),
                           np.arange(c1 * CH, (c1 + 1) * CH)])


def build_masks(parity: int):
    """Multiplicative 0/1 masks in the P-tile layout."""
    k_idx = np.arange(128)
    q_idx = np.arange(CH)

    def blk(g, kt, qs):
        lk = GORDER[g]
        lq = QCH[parity][qs]
        kk = lk * CH + kt * 128 + k_idx[:, None]
        qq = lq * CH + q_idx[None, :]
        return (kk <= qq).astype(np.float32)

    maska = np.stack([blk(0, 0, 0), blk(0, 1, 0), blk(2, 0, 0), blk(2, 1, 0)],
                     axis=1)
    maskb = np.concatenate(
        [blk(1, 0, 1), blk(1, 1, 1), blk(3, 0, 1), blk(3, 1, 1)], axis=1)
    return np.ascontiguousarray(maska), np.ascontiguousarray(maskb)


_NC_CACHE = None


def prepare_in_maps(inputs):
    toks = np.asarray(inputs["toks"], np.float32)
    pos_W = np.asarray(inputs["pos_W"], np.float32)
    attn_W = np.asarray(inputs["attn_W"], np.float32)

    aw = attn_W.reshape(L, E, H, 3 * C)
    shared = {
        "wqk": rne12(np.ascontiguousarray(aw[:, :, :, 0:2 * C])),
        "wv": rne12(np.ascontiguousarray(aw[:, :, :, 2 * C:]).reshape(L, E, H * C)),
        "w1": rne12(np.asarray(inputs["mlp_W1"], np.float32)),
        "w2": rne12(np.asarray(inputs["mlp_W2"], np.float32)),
        "b1": np.ascontiguousarray(inputs["mlp_b1"], np.float32),
        "b2": rne12(np.asarray(inputs["mlp_b2"], np.float32)),
        "ln1g": np.ascontiguousarray(inputs["ln1_g"], np.float32),
        "ln1b": np.ascontiguousarray(inputs["ln1_b"], np.float32),
        "ln2g": np.ascontiguousarray(inputs["ln2_g"], np.float32),
        "ln2b": np.ascontiguousarray(inputs["ln2_b"], np.float32),
        "lnfg": np.ascontiguousarray(inputs["lnf_g"], np.float32),
        "lnfb": np.ascontiguousarray(inputs["lnf_b"], np.float32),
        "tokw": rne12(np.asarray(inputs["tok_W"], np.float32)),
        "uw": rne12(np.asarray(inputs["unembed_W"], np.float32)),
        "ub": rne12(np.asarray(inputs["unembed_b"], np.float32)),
    }
    in_maps = []
    for c in range(NC):
        b, p = c // 2, c % 2
        idx = core_token_idx(c)
        ma, mb = build_masks(p)
        m = dict(shared)
        m["toksT"] = rne12(np.ascontiguousarray(toks[b, idx, :].T))
        m["posT"] = np.ascontiguousarray(pos_W[idx, :].T)
        m["maska"] = ma
        m["maskb"] = mb
        m["pboff"] = np.array([[(c // 2) * 2 * BLOB]], dtype=np.uint32)
        in_maps.append(m)
    return in_maps


def kernel(**inputs) -> np.ndarray:
    global _NC_CACHE
    if _NC_CACHE is None:
        _NC_CACHE = build_program()
    nc = _NC_CACHE
    in_maps = prepare_in_maps(inputs)

    r = run_bass_kernel_spmd(nc, in_maps, core_ids=list(range(NC)))

    out = np.empty((B, T, V), np.float32)
    for c in range(NC):
        b = c // 2
        idx = core_token_idx(c)
        out[b, idx, :] = r.results[c]["logits"].T
    return out


if __name__ == "__main__":
    print("building program...")
    nc0 = build_program()
    print("built ok")


# revision 8
# speedup vs baseline: 25.6150x; 25.6150x over previous
# Trainium2 Bass kernel for an 8-layer dense transformer (B=4, T=1024,
# V=E=1024, H=16, M=4096), 8-way SPMD across one chip.
#
# Sharding: data-parallel over (batch x 2 interleaved token chunks) ->
# 8 shards of 512 tokens.  Even cores own logical 256-token chunks (0,3)
# of their batch, odd cores own (1,2), which balances causal-attention
# work.  Per layer, each core projects Q/K/V for its own tokens, local
# K/V are exchanged through a single 8-rank AllGather, and everything
# else (LN, QKV, attention, MLP) runs locally with replicated weights.
#
# Layouts: the residual stream is feature-major ([E on partitions,
# tokens on free dim]) so every GEMM consumes the natural row-major
# weight layout with zero transposes.  Scores are key-major so the PV
# matmul is native; the softmax denominator comes out of the same PV
# matmul via a 65th all-ones column appended to V; softmax skips the
# max-subtraction (logits are O(1) here).  Causal masking is a per-core
# 0/1 multiplicative mask tensor so the compiled program is identical
# on all 8 cores; cross-partition LN stats/broadcasts go through tiny
# ones-matmuls on the tensor engine.
#
# All matmuls run in float32r (fp32 storage, ~11 mantissa bits, full PE
# rate at N>=256).  Weights are pre-rounded to fp32r on the host (RNE
# on the low 12 bits) and DMA'd directly into fp32r SBUF tiles.

import os
import sys

for _p in ("/opt/trn_rl_repo", "/root/.axon_site/_ro/trn_rl_repo"):
    if _p not in sys.path and os.path.isdir(_p):
        sys.path.insert(0, _p)

import numpy as np

import concourse.bass as bass
import concourse.mybir as mybir
import concourse.tile as tile
from concourse import bacc
from concourse.bass_utils import run_bass_kernel_spmd

B, T, V, E, H, M, L, C = 4, 1024, 1024, 1024, 16, 4096, 8, 64
NC = 8          # cores
S = 512         # tokens per core
CH = 256        # chunk size
EPS = 1e-5
SCALE = 1.0 / 8.0   # 1/sqrt(C)

F32 = mybir.dt.float32
F32R = mybir.dt.float32r
AOT = mybir.AluOpType
AFT = mybir.ActivationFunctionType

N_LAYERS = int(os.environ.get("KERNEL_LAYERS", str(L)))
DEBUG_X = bool(int(os.environ.get("KERNEL_DEBUG_X", "0")))
# Replace the collective with local DMA copies and build single-core —
# for offline timeline-simulation only.
FAKE_AG = bool(int(os.environ.get("KERNEL_FAKE_AG", "0")))

# Gathered key-chunk slot -> logical chunk (AG concat: even core's
# chunks (0,3) then odd core's (1,2)).
GORDER = [0, 3, 1, 2]
# Core parity -> logical chunks of its two local q-slots.
QCH = [(0, 3), (1, 2)]

BLOB = S * 2 * E          # 1048576 elems per rank blob (K + V)
VOFF = 8 * 2 * C * S      # 524288; V region: [512 rows, 1024] rows=local token

# P-tile column base for each (g, kt) score block.
PCOL = {(0, 0): 0, (0, 1): 512, (2, 0): 1024, (2, 1): 1536,
        (1, 0): 2048, (1, 1): 2304, (3, 0): 2560, (3, 1): 2816}
BLK512 = [(0, 0), (0, 1), (2, 0), (2, 1)]   # N=512 (both q slots)
BLK256 = [(1, 0), (1, 1), (3, 0), (3, 1)]   # N=256 (q slot 1 only)
# PV accumulation order: first/last must be full-width (N=512) blocks.
PV_ORDER = [(0, 0), (1, 0), (1, 1), (3, 0), (3, 1), (0, 1), (2, 0), (2, 1)]


def rne12(x: np.ndarray) -> np.ndarray:
    """Round fp32 to float32r (round-to-nearest-even, drop low 12 bits)."""
    x = np.ascontiguousarray(x, dtype=np.float32)
    u = x.view(np.uint32).astype(np.uint64)
    lsb = (u >> np.uint64(12)) & np.uint64(1)
    u2 = (u + np.uint64(0x7FF) + lsb) >> np.uint64(12) << np.uint64(12)
    return u2.astype(np.uint32).view(np.float32).reshape(x.shape)


def build_program():
    nc = bacc.Bacc("TRN2", target_bir_lowering=False, debug=False,
                   num_devices=(1 if FAKE_AG else NC))

    # ---- DRAM I/O ----
    d_toksT = nc.dram_tensor("toksT", [V, S], F32R, kind="ExternalInput")
    d_posT = nc.dram_tensor("posT", [E, S], F32, kind="ExternalInput")
    d_maska = nc.dram_tensor("maska", [128, 4, CH], F32, kind="ExternalInput")
    d_maskb = nc.dram_tensor("maskb", [128, 4 * CH], F32, kind="ExternalInput")
    d_pboff = nc.dram_tensor("pboff", [1, 1], mybir.dt.uint32, kind="ExternalInput")
    d_wqk = nc.dram_tensor("wqk", [L, E, H, 2 * C], F32R, kind="ExternalInput")
    d_wv = nc.dram_tensor("wv", [L, E, H * C], F32R, kind="ExternalInput")
    d_w1 = nc.dram_tensor("w1", [L, E, M], F32R, kind="ExternalInput")
    d_w2 = nc.dram_tensor("w2", [L, M, E], F32R, kind="ExternalInput")
    d_b1 = nc.dram_tensor("b1", [L, M], F32, kind="ExternalInput")
    d_b2 = nc.dram_tensor("b2", [L, E], F32R, kind="ExternalInput")
    d_ln1g = nc.dram_tensor("ln1g", [L, E], F32, kind="ExternalInput")
    d_ln1b = nc.dram_tensor("ln1b", [L, E], F32, kind="ExternalInput")
    d_ln2g = nc.dram_tensor("ln2g", [L, E], F32, kind="ExternalInput")
    d_ln2b = nc.dram_tensor("ln2b", [L, E], F32, kind="ExternalInput")
    d_lnfg = nc.dram_tensor("lnfg", [E], F32, kind="ExternalInput")
    d_lnfb = nc.dram_tensor("lnfb", [E], F32, kind="ExternalInput")
    d_tokw = nc.dram_tensor("tokw", [V, E], F32R, kind="ExternalInput")
    d_uw = nc.dram_tensor("uw", [E, V], F32R, kind="ExternalInput")
    d_ub = nc.dram_tensor("ub", [V], F32R, kind="ExternalInput")
    d_out = nc.dram_tensor("logits", [V, S], F32, kind="ExternalOutput")

    with tile.TileContext(nc) as tc:
        from contextlib import ExitStack
        top = ExitStack()
        pers = top.enter_context(tc.tile_pool(name="pers", bufs=1))
        small = top.enter_context(tc.tile_pool(name="small", bufs=1))
        stage = top.enter_context(tc.tile_pool(name="stage", bufs=2))
        bigpool = top.enter_context(tc.tile_pool(name="bigpool", bufs=1))
        dram = top.enter_context(tc.tile_pool(name="dram", bufs=2, space="DRAM"))
        dram_sh = top.enter_context(tc.tile_pool(name="dram_sh", bufs=2, space="DRAM"))

        # ---- persistent SBUF state ----
        x = pers.tile([128, 8, S], F32)        # residual (feature-major)
        inner = pers.tile([128, 8, S], F32)    # x + attn_out (+ mlp out)
        h = pers.tile([128, 8, S], F32R)       # LN output / rounded x
        qsb = pers.tile([128, 8, S], F32R)     # Q (head-pair-major)
        maska = pers.tile([128, 4, CH], F32)
        maskb = pers.tile([128, 4 * CH], F32)
        ones_col = pers.tile([128, 1], F32R)   # stats lhsT
        ones_row = pers.tile([1, 128], F32R)   # broadcast lhsT
        neg_row = pers.tile([1, 128], F32R)
        ones_s = pers.tile([1, S], F32R)       # bias outer-product rhs
        g1 = pers.tile([128, 8], F32)
        bb1 = pers.tile([128, 8], F32)
        g2 = pers.tile([128, 8], F32)
        bb2 = pers.tile([128, 8], F32)
        gf = pers.tile([128, 8], F32)
        bf = pers.tile([128, 8], F32)
        b1sb = pers.tile([128, 32], F32)
        b2r = pers.tile([1, E], F32R)
        ubr = pers.tile([1, V], F32R)
        lnst = small.tile([1, 4 * S], F32)     # mu | msq | var | rvar
        lnst_r = small.tile([1, 2 * S], F32R)  # rstd | nmu

        nc.vector.memset(ones_col[:].bitcast(F32), 1.0)
        nc.vector.memset(ones_row[:].bitcast(F32), 1.0)
        nc.vector.memset(neg_row[:].bitcast(F32), -1.0)
        nc.vector.memset(ones_s[:].bitcast(F32), 1.0)
        nc.sync.dma_start(maska[:], d_maska[:])
        nc.sync.dma_start(maskb[:], d_maskb[:])
        nc.sync.dma_start(gf[:], d_lnfg[:].rearrange("(o p) -> p o", p=128))
        nc.sync.dma_start(bf[:], d_lnfb[:].rearrange("(o p) -> p o", p=128))
        nc.sync.dma_start(ubr[:], d_ub[None, :])

        # pair-base register: element offset of my pair's even-rank blob
        # in the AllGather output.
        if FAKE_AG:
            pb = 0
        else:
            pbreg_t = nc.sync.alloc_register("pbreg")
            nc.sync.reg_load(pbreg_t, d_pboff[0:1, 0:1])
            pb = nc.sync.snap(pbreg_t, donate=True, min_val=0, max_val=6 * BLOB)

        # ---------------- helpers ----------------
        def layer_norm(src, g_ap, b_ap):
            """src: [128,8,S] F32 sbuf -> h (F32R).  h doubles as the
            rounded copy of src that feeds the stats matmuls."""
            mu = lnst[:, 0:S]
            msq = lnst[:, S:2 * S]
            var = lnst[:, 2 * S:3 * S]
            rvar = lnst[:, 3 * S:4 * S]
            rstd = lnst_r[:, 0:S]
            nmu = lnst_r[:, S:2 * S]
            with tc.tile_pool(name="ps_ln", bufs=2, space="PSUM") as ps_ln:
                for t in range(8):
                    nc.vector.tensor_copy(h[:, t, :], src[:, t, :])
                st_sum = ps_ln.tile([1, S], F32, tag="stat")
                st_sq = ps_ln.tile([1, S], F32, tag="stat")
                for t in range(8):
                    nc.tensor.matmul(st_sum[:], ones_col[:], h[:, t, :],
                                     start=(t == 0), stop=(t == 7))
                for t in range(8):
                    sqt = stage.tile([128, S], F32R, tag="sqt", bufs=2)
                    nc.vector.tensor_tensor(sqt[:], src[:, t, :], src[:, t, :],
                                            AOT.mult)
                    nc.tensor.matmul(st_sq[:], ones_col[:], sqt[:],
                                     start=(t == 0), stop=(t == 7))
                nc.vector.tensor_scalar_mul(mu, st_sum[:], 1.0 / E)
                nc.vector.tensor_scalar_mul(msq, st_sq[:], 1.0 / E)
                nc.vector.tensor_tensor(var, mu, mu, AOT.mult)
                nc.vector.tensor_tensor(var, msq, var, AOT.subtract)
                nc.vector.tensor_scalar_add(var, var, EPS)
                nc.vector.reciprocal(rvar, var)
                with nc.allow_low_precision(reason="fp32r rstd for matmul bcast"):
                    nc.scalar.sqrt(rstd, rvar)
                nc.vector.tensor_tensor(nmu, mu, rstd, AOT.mult)
                bc_rstd = ps_ln.tile([128, S], F32, tag="bc")
                bc_nmu = ps_ln.tile([128, S], F32, tag="bc")
                nc.tensor.matmul(bc_rstd[:], ones_row[:], rstd,
                                 start=True, stop=True)
                nc.tensor.matmul(bc_nmu[:], neg_row[:], nmu,
                                 start=True, stop=True)
                for t in range(8):
                    tmp = stage.tile([128, S], F32, tag="lntmp", bufs=2)
                    nc.vector.tensor_tensor(tmp[:], src[:, t, :], bc_rstd[:],
                                            AOT.mult)
                    nc.vector.tensor_tensor(tmp[:], tmp[:], bc_nmu[:], AOT.add)
                    nc.vector.tensor_scalar(h[:, t, :], tmp[:],
                                            g_ap[:, t:t + 1], b_ap[:, t:t + 1],
                                            AOT.mult, AOT.add)

        # ---------------- embedding ----------------
        with tc.tile_pool(name="emb", bufs=1) as emb, \
             tc.tile_pool(name="wemb", bufs=2) as wemb, \
             tc.tile_pool(name="ps_emb", bufs=4, space="PSUM") as ps_emb:
            toksr = emb.tile([128, 8, S], F32R)
            for v in range(8):
                nc.sync.dma_start(toksr[:, v, :], d_toksT[v * 128:(v + 1) * 128, :])
            for eg in range(4):
                psl = [ps_emb.tile([128, S], F32, tag="embps", name=f"embps{_i}") for _i in range(2)]
                for vh in range(4):
                    tw = wemb.tile([128, 2, 256], F32R, tag="twt")
                    for v2 in range(2):
                        nc.sync.dma_start(
                            tw[:, v2, :],
                            d_tokw[(vh * 2 + v2) * 128:(vh * 2 + v2 + 1) * 128,
                                   eg * 256:(eg + 1) * 256])
                    for v2 in range(2):
                        v = vh * 2 + v2
                        for eo2 in range(2):
                            eo = eg * 2 + eo2
                            nc.tensor.matmul(
                                psl[eo2][:],
                                tw[:, v2, eo2 * 128:(eo2 + 1) * 128],
                                toksr[:, v, :],
                                start=(v == 0), stop=(v == 7))
                for eo2 in range(2):
                    nc.scalar.copy(x[:, eg * 2 + eo2, :], psl[eo2][:])
        with tc.tile_pool(name="embp", bufs=1) as embp:
            possb = embp.tile([128, 8, S], F32)
            for e in range(8):
                nc.sync.dma_start(possb[:, e, :], d_posT[e * 128:(e + 1) * 128, :])
                nc.vector.tensor_tensor(x[:, e, :], x[:, e, :], possb[:, e, :],
                                        AOT.add)

        # ---------------- layers ----------------
        for l in range(N_LAYERS):
            nc.sync.dma_start(g1[:], d_ln1g[l].rearrange("(o p) -> p o", p=128))
            nc.sync.dma_start(bb1[:], d_ln1b[l].rearrange("(o p) -> p o", p=128))
            nc.sync.dma_start(g2[:], d_ln2g[l].rearrange("(o p) -> p o", p=128))
            nc.sync.dma_start(bb2[:], d_ln2b[l].rearrange("(o p) -> p o", p=128))
            nc.sync.dma_start(b1sb[:], d_b1[l].rearrange("(o p) -> p o", p=128))
            nc.sync.dma_start(b2r[:], d_b2[l][None, :])

            ag_in = dram.tile([BLOB], F32R, tag="agin")
            ag_out = dram_sh.tile([NC * BLOB], F32R, tag="agout",
                                  addr_space=("Local" if FAKE_AG else "Shared"))
            agi_k = ag_in[:].rearrange("(r c) -> r c", c=S)   # [1024, 512]

            # ---- LN1 ----
            layer_norm(x, g1, bb1)

            # ---- QKV projection (Q,K feature-major; K straight to DRAM) ----
            with tc.tile_pool(name="wqkv", bufs=2) as wqkv, \
                 tc.tile_pool(name="ps_qkv", bufs=8, space="PSUM") as ps_qkv:
                for hg in range(2):
                    qk_ps = [ps_qkv.tile([128, S], F32, tag="qkps", bufs=8,
                                         name=f"qkps{_i}") for _i in range(8)]
                    for e in range(8):
                        wq = wqkv.tile([128, 8, 128], F32R, tag="wqk", bufs=2)
                        nc.sync.dma_start(
                            wq[:], d_wqk[l, e * 128:(e + 1) * 128,
                                         hg * 8:(hg + 1) * 8, :])
                        for hh in range(8):
                            nc.tensor.matmul(qk_ps[hh][:], wq[:, hh, :],
                                             h[:, e, :],
                                             start=(e == 0), stop=(e == 7))
                    for hh in range(8):
                        ha = hg * 8 + hh
                        hp, par = ha // 2, ha % 2
                        nc.scalar.copy(qsb[par * 64:par * 64 + 64, hp, :],
                                       qk_ps[hh][0:64, :])
                        kst = stage.tile([64, S], F32R, tag="kst", bufs=3)
                        nc.scalar.copy(kst[:], qk_ps[hh][64:128, :])
                        nc.sync.dma_start(
                            agi_k[hp * 128 + par * 64: hp * 128 + par * 64 + 64,
                                  :],
                            kst[:])
                # ---- V projection (token-major) ----
                v_ps = [[ps_qkv.tile([128, 512], F32, tag="qkps", bufs=8,
                                     name=f"vps{_i}_{_j}")
                         for _j in range(2)] for _i in range(4)]
                for e in range(8):
                    wv = wqkv.tile([128, 1024], F32R, tag="wv", bufs=2)
                    nc.sync.dma_start(wv[:], d_wv[l, e * 128:(e + 1) * 128, :])
                    for tt in range(4):
                        for hf in range(2):
                            nc.tensor.matmul(
                                v_ps[tt][hf][:],
                                h[:, e, tt * 128:(tt + 1) * 128],
                                wv[:, hf * 512:(hf + 1) * 512],
                                start=(e == 0), stop=(e == 7))
                for tt in range(4):
                    vst = stage.tile([128, 1024], F32R, tag="vst", bufs=2)
                    nc.scalar.copy(vst[:, 0:512], v_ps[tt][0][:])
                    nc.scalar.copy(vst[:, 512:1024], v_ps[tt][1][:])
                    nc.sync.dma_start(
                        ag_in[VOFF + tt * 128 * 1024: VOFF + (tt + 1) * 128 * 1024]
                        .rearrange("(r c) -> r c", c=1024),
                        vst[:])

            # ---- KV exchange ----
            if FAKE_AG:
                nc.sync.dma_start(ag_out[0:BLOB], ag_in[:])
                nc.sync.dma_start(ag_out[BLOB:2 * BLOB], ag_in[:])
            else:
                nc.gpsimd.collective_compute(
                    "AllGather", AOT.bypass,
                    replica_groups=[list(range(NC))],
                    ins=[ag_in.opt()], outs=[ag_out.opt()])

            # gathered V (token-major, with interleaved ones column)
            vgr = bigpool.tile([128, 8, H, C + 1], F32R, tag="vgrm", bufs=1)
            for kt in range(8):
                blob = pb if kt < 4 else pb + BLOB
                src = ag_out[bass.ds(blob + VOFF + (kt % 4) * 128 * 1024,
                                     128 * 1024)].rearrange(
                    "(r hh cc) -> r hh cc", hh=H, cc=C)
                nc.sync.dma_start(vgr[:, kt, :, 0:C], src)
            nc.vector.memset(vgr[:, :, :, C:C + 1].bitcast(F32), 1.0)

            # ---- attention ----
            with tc.tile_pool(name="attnp", bufs=1) as attnp, \
                 tc.tile_pool(name="ps_s", bufs=4, space="PSUM") as ps_s, \
                 tc.tile_pool(name="ps_y", bufs=2, space="PSUM") as ps_y, \
                 tc.tile_pool(name="ps_by", bufs=2, space="PSUM") as ps_by:
                for hp in range(8):
                    kr = attnp.tile([128, 1024], F32R, tag="kr", bufs=2)
                    nc.sync.dma_start(
                        kr[:, 0:512],
                        ag_out[bass.ds(pb + hp * 128 * S, 128 * S)]
                        .rearrange("(r c) -> r c", c=S))
                    nc.sync.dma_start(
                        kr[:, 512:1024],
                        ag_out[bass.ds(pb + BLOB + hp * 128 * S, 128 * S)]
                        .rearrange("(r c) -> r c", c=S))
                    for head in range(2):
                        ha = hp * 2 + head
                        qb = head * 64
                        P = attnp.tile([128, 3072], F32R, tag="P", bufs=1)
                        for (g, kt) in BLK512:
                            ktg = g * 2 + kt
                            sps = ps_s.tile([128, 512], F32, tag="S", bufs=4)
                            nc.tensor.matmul(
                                sps[:],
                                kr[qb:qb + 64, ktg * 128:(ktg + 1) * 128],
                                qsb[qb:qb + 64, hp, :],
                                start=True, stop=True)
                            pc = PCOL[(g, kt)]
                            nc.scalar.activation(P[:, pc:pc + 512], sps[:],
                                                 AFT.Exp, scale=SCALE)
                        for (g, kt) in BLK256:
                            ktg = g * 2 + kt
                            sps = ps_s.tile([128, 256], F32, tag="S", bufs=4)
                            nc.tensor.matmul(
                                sps[:],
                                kr[qb:qb + 64, ktg * 128:(ktg + 1) * 128],
                                qsb[qb:qb + 64, hp, 256:512],
                                start=True, stop=True)
                            pc = PCOL[(g, kt)]
                            nc.scalar.activation(P[:, pc:pc + 256], sps[:],
                                                 AFT.Exp, scale=SCALE)
                        # causal mask (0/1 multiplicative)
                        ap1 = P[:, 0:2048].rearrange(
                            "p (b q) -> p b q", q=512)[:, :, 0:CH]
                        nc.vector.tensor_tensor(ap1, ap1, maska[:], AOT.mult)
                        ap2 = P[:, 2048:3072]
                        nc.vector.tensor_tensor(ap2, ap2, maskb[:], AOT.mult)
                        # PV (+ denominator via ones column)
                        yps = ps_y.tile([128, 512], F32, tag="y", bufs=2)
                        for i, (g, kt) in enumerate(PV_ORDER):
                            ktg = g * 2 + kt
                            pc = PCOL[(g, kt)]
                            n = 512 if (g, kt) in BLK512 else 256
                            qoff = 0 if n == 512 else 256
                            nc.tensor.matmul(
                                yps[0:65, qoff:qoff + n],
                                vgr[:, ktg, ha, :],
                                P[:, pc:pc + n],
                                start=(i == 0), stop=(i == len(PV_ORDER) - 1))
                        rd = small.tile([1, S], F32R, tag="rd", bufs=2)
                        with nc.allow_low_precision(reason="fp32r 1/d for bcast"):
                            nc.vector.reciprocal(rd[:], yps[64:65, :])
                        bcd = ps_by.tile([64, S], F32, tag="bcd", bufs=2)
                        nc.tensor.matmul(bcd[:], ones_row[:, 0:64], rd[:],
                                         start=True, stop=True)
                        et, ebase = ha // 2, (ha % 2) * 64
                        ysb = stage.tile([128, S], F32, tag="ysb", bufs=2)
                        ysl = ysb[ebase:ebase + 64, :]
                        nc.scalar.copy(ysl, yps[0:64, :])
                        nc.vector.tensor_tensor(ysl, ysl, bcd[:], AOT.mult)
                        # inner = x + y  (y slice: feature rows of head ha)
                        nc.vector.tensor_tensor(
                            inner[ebase:ebase + 64, et, :],
                            x[ebase:ebase + 64, et, :], ysl, AOT.add)

            # ---- LN2 + MLP (mlp out accumulates into `inner`) ----
            layer_norm(inner, g2, bb2)
            with tc.tile_pool(name="wmlp", bufs=1) as wmlp, \
                 tc.tile_pool(name="ps_m", bufs=5, space="PSUM") as ps_m, \
                 tc.tile_pool(name="ps_o", bufs=3, space="PSUM") as ps_o:
                for half in range(2):
                    m_sb = bigpool.tile([128, 16, 512], F32R, tag="vgrm", bufs=1)
                    for og4 in range(4):
                        og = half * 4 + og4
                        mps_l = [ps_m.tile([128, S], F32, tag="mps", bufs=5,
                                           name=f"mps{_i}") for _i in range(4)]
                        for eh in range(2):
                            w1h = wmlp.tile([128, 4, 512], F32R, tag="w1h",
                                            bufs=2)
                            for e4 in range(4):
                                e = eh * 4 + e4
                                nc.sync.dma_start(
                                    w1h[:, e4, :],
                                    d_w1[l, e * 128:(e + 1) * 128,
                                         og * 512:(og + 1) * 512])
                            for ob in range(4):
                                for e4 in range(4):
                                    e = eh * 4 + e4
                                    nc.tensor.matmul(
                                        mps_l[ob][:],
                                        w1h[:, e4, ob * 128:(ob + 1) * 128],
                                        h[:, e, :],
                                        start=(e == 0), stop=(e == 7))
                        for ob in range(4):
                            mtl = og4 * 4 + ob
                            mt_abs = half * 16 + mtl
                            nc.scalar.activation(
                                m_sb[:, mtl, :], mps_l[ob][:], AFT.Relu,
                                bias=b1sb[:, mt_abs:mt_abs + 1], scale=1.0)
                    for eop in range(4):
                        ops = [ps_o.tile([128, S], F32, tag="ops", bufs=3,
                                         name=f"ops{_i}") for _i in range(2)]
                        if half == 0:
                            for eo2 in range(2):
                                eo = eop * 2 + eo2
                                nc.tensor.matmul(
                                    ops[eo2][:],
                                    b2r[:, eo * 128:(eo + 1) * 128],
                                    ones_s[:], start=True, stop=False)
                        for mt in range(16):
                            mt_abs = half * 16 + mt
                            w2t = wmlp.tile([128, 256], F32R, tag="w2", bufs=3)
                            nc.sync.dma_start(
                                w2t[:], d_w2[l, mt_abs * 128:(mt_abs + 1) * 128,
                                             eop * 256:(eop + 1) * 256])
                            for eo2 in range(2):
                                nc.tensor.matmul(
                                    ops[eo2][:],
                                    w2t[:, eo2 * 128:(eo2 + 1) * 128],
                                    m_sb[:, mt, :],
                                    start=(half == 1 and mt == 0),
                                    stop=(mt == 15))
                        for eo2 in range(2):
                            eo = eop * 2 + eo2
                            nc.vector.tensor_tensor(inner[:, eo, :],
                                                    inner[:, eo, :],
                                                    ops[eo2][:], AOT.add)
            # x = x + inner  (inner now holds x + attn + mlp + b2)
            for e in range(8):
                nc.vector.tensor_tensor(x[:, e, :], x[:, e, :], inner[:, e, :],
                                        AOT.add)

        # ---------------- final LN + unembed ----------------
        if DEBUG_X:
            for e in range(8):
                xs = stage.tile([128, S], F32, tag="lntmp", bufs=2)
                nc.vector.tensor_copy(xs[:], x[:, e, :])
                nc.sync.dma_start(d_out[e * 128:(e + 1) * 128, :], xs[:])
        else:
            layer_norm(x, gf, bf)
            with tc.tile_pool(name="wu", bufs=2) as wu, \
                 tc.tile_pool(name="ps_u", bufs=5, space="PSUM") as ps_u:
                for vg in range(2):
                    upl = [ps_u.tile([128, S], F32, tag="ups", bufs=5,
                                     name=f"ups{_i}") for _i in range(4)]
                    for vo4 in range(4):
                        vo = vg * 4 + vo4
                        nc.tensor.matmul(upl[vo4][:],
                                         ubr[:, vo * 128:(vo + 1) * 128],
                                         ones_s[:], start=True, stop=False)
                    for eh in range(2):
                        uwh = wu.tile([128, 4, 512], F32R, tag="uwh", bufs=2)
                        for e4 in range(4):
                            e = eh * 4 + e4
                            nc.sync.dma_start(
                                uwh[:, e4, :],
                                d_uw[e * 128:(e + 1) * 128,
                                     vg * 512:(vg + 1) * 512])
                        for vo4 in range(4):
                            for e4 in range(4):
                                e = eh * 4 + e4
                                nc.tensor.matmul(
                                    upl[vo4][:],
                                    uwh[:, e4, vo4 * 128:(vo4 + 1) * 128],
                                    h[:, e, :], start=False, stop=(e == 7))
                    for vo4 in range(4):
                        vo = vg * 4 + vo4
                        lst = stage.tile([128, S], F32, tag="lntmp", bufs=2)
                        nc.scalar.copy(lst[:], upl[vo4][:])
                        nc.sync.dma_start(d_out[vo * 128:(vo + 1) * 128, :],
                                          lst[:])

        top.close()

    nc.compile()
    return nc


def core_token_idx(c: int) -> np.ndarray:
    p = c % 2
    c0, c1 = QCH[p]
    return np.concatenate([np.arange(c0 * CH, (c0 + 1) * CH),
                           np.arange(c1 * CH, (c1 + 1) * CH)])


def build_masks(parity: int):
    """Multiplicative 0/1 masks in the P-tile layout."""
    k_idx = np.arange(128)
    q_idx = np.arange(CH)

    def blk(g, kt, qs):
        lk = GORDER[g]
        lq = QCH[parity][qs]
        kk = lk * CH + kt * 128 + k_idx[:, None]
        qq = lq * CH + q_idx[None, :]
        return (kk <= qq).astype(np.float32)

    maska = np.stack([blk(0, 0, 0), blk(0, 1, 0), blk(2, 0, 0), blk(2, 1, 0)],
                     axis=1)
    maskb = np.concatenate(
        [blk(1, 0, 1), blk(1, 1, 1), blk(3, 0, 1), blk(3, 1, 1)], axis=1)
    return np.ascontiguousarray(maska), np.ascontiguousarray(maskb)


_NC_CACHE = None


def prepare_in_maps(inputs):
    toks = np.asarray(inputs["toks"], np.float32)
    pos_W = np.asarray(inputs["pos_W"], np.float32)
    attn_W = np.asarray(inputs["attn_W"], np.float32)

    aw = attn_W.reshape(L, E, H, 3 * C)
    shared = {
        "wqk": rne12(np.ascontiguousarray(aw[:, :, :, 0:2 * C])),
        "wv": rne12(np.ascontiguousarray(aw[:, :, :, 2 * C:]).reshape(L, E, H * C)),
        "w1": rne12(np.asarray(inputs["mlp_W1"], np.float32)),
        "w2": rne12(np.asarray(inputs["mlp_W2"], np.float32)),
        "b1": np.ascontiguousarray(inputs["mlp_b1"], np.float32),
        "b2": rne12(np.asarray(inputs["mlp_b2"], np.float32)),
        "ln1g": np.ascontiguousarray(inputs["ln1_g"], np.float32),
        "ln1b": np.ascontiguousarray(inputs["ln1_b"], np.float32),
        "ln2g": np.ascontiguousarray(inputs["ln2_g"], np.float32),
        "ln2b": np.ascontiguousarray(inputs["ln2_b"], np.float32),
        "lnfg": np.ascontiguousarray(inputs["lnf_g"], np.float32),
        "lnfb": np.ascontiguousarray(inputs["lnf_b"], np.float32),
        "tokw": rne12(np.asarray(inputs["tok_W"], np.float32)),
        "uw": rne12(np.asarray(inputs["unembed_W"], np.float32)),
        "ub": rne12(np.asarray(inputs["unembed_b"], np.float32)),
    }
    in_maps = []
    for c in range(NC):
        b, p = c // 2, c % 2
        idx = core_token_idx(c)
        ma, mb = build_masks(p)
        m = dict(shared)
        m["toksT"] = rne12(np.ascontiguousarray(toks[b, idx, :].T))
        m["posT"] = np.ascontiguousarray(pos_W[idx, :].T)
        m["maska"] = ma
        m["maskb"] = mb
        m["pboff"] = np.array([[(c // 2) * 2 * BLOB]], dtype=np.uint32)
        in_maps.append(m)
    return in_maps


def kernel(**inputs) -> np.ndarray:
    global _NC_CACHE
    if _NC_CACHE is None:
        _NC_CACHE = build_program()
    nc = _NC_CACHE
    in_maps = prepare_in_maps(inputs)

    r = run_bass_kernel_spmd(nc, in_maps, core_ids=list(range(NC)))

    out = np.empty((B, T, V), np.float32)
    for c in range(NC):
        b = c // 2
        idx = core_token_idx(c)
        out[b, idx, :] = r.results[c]["logits"].T
    return out


if __name__ == "__main__":
    print("building program...")
    nc0 = build_program()
    print("built ok")


# revision 16
# speedup vs baseline: 27126.6207x; 1059.0134x over previous
# Trainium2 Bass kernel for an 8-layer dense transformer (B=4, T=1024,
# V=E=1024, H=16, M=4096), 8-way SPMD across one chip.
#
# Sharding: data-parallel over (batch x 2 interleaved token chunks) ->
# 8 shards of 512 tokens.  Even cores own logical 256-token chunks (0,3)
# of their batch, odd cores own (1,2), which balances causal-attention
# work.  Per layer each core projects Q/K/V for its own tokens; local
# K and V are exchanged through two 8-rank AllGathers (V's gather
# overlaps the Q/K projection); everything else runs locally with
# replicated weights.
#
# Layouts: the residual stream is feature-major ([E on partitions,
# tokens free]) so every GEMM consumes the natural row-major weight
# layout with zero transposes.  Scores are key-major so the PV matmul
# is native; the softmax denominator comes out of the same PV matmul
# via a 65th all-ones column appended to V; softmax skips the max
# subtraction (logits are O(1) here).  Causal masking is a per-core
# 0/1 multiplicative bf16 mask so the program is identical on all
# cores; cross-partition LN stats/broadcasts go through tiny
# ones-matmuls on the tensor engine, with the LN gain/bias folded into
# rank-2 outer-product broadcasts.
#
# Big GEMMs run in float32r (fp32 storage, ~11 mantissa bits, full PE
# rate at N>=256; weights pre-rounded on the host so plain DMA feeds
# the matmuls).  Attention operands (Q, K, V, P) are bf16.

import os
import sys

for _p in ("/opt/trn_rl_repo", "/root/.axon_site/_ro/trn_rl_repo"):
    if _p not in sys.path and os.path.isdir(_p):
        sys.path.insert(0, _p)

import numpy as np

import concourse.bass as bass
import concourse.mybir as mybir
import concourse.tile as tile
from concourse import bacc
from concourse.bass_utils import run_bass_kernel_spmd

B, T, V, E, H, M, L, C = 4, 1024, 1024, 1024, 16, 4096, 8, 64
NC = 8          # cores
S = 512         # tokens per core
CH = 256        # chunk size
EPS = 1e-5
SCALE = 1.0 / 8.0   # 1/sqrt(C)

F32 = mybir.dt.float32
F32R = mybir.dt.float32r
BF16 = mybir.dt.bfloat16
AOT = mybir.AluOpType
AFT = mybir.ActivationFunctionType

N_LAYERS = int(os.environ.get("KERNEL_LAYERS", str(L)))
DEBUG_X = bool(int(os.environ.get("KERNEL_DEBUG_X", "0")))
# Replace collectives with local DMA copies (timing experiments).
FAKE_AG = bool(int(os.environ.get("KERNEL_FAKE_AG", "0")))
# Build single-core (offline timeline-simulation only).
SINGLE = bool(int(os.environ.get("KERNEL_SINGLE", "0")))
# Repeat the computation R times in a hardware loop (timing).
HWLOOP = int(os.environ.get("KERNEL_HWLOOP", "1"))
if SINGLE or HWLOOP > 1:
    FAKE_AG = True

# Gathered key-chunk slot -> logical chunk (AG concat: even core's
# chunks (0,3) then odd core's (1,2)).
GORDER = [0, 3, 1, 2]
# Core parity -> logical chunks of its two local q-slots.
QCH = [(0, 3), (1, 2)]

HBLOB = S * E             # 524288 elems per rank half-blob (V), bf16
QBLOB = S * E // 2       # 262144 elems per rank quarter blob (half of K)

# P-tile column base for each (g, kt) score block.
PCOL = {(0, 0): 0, (0, 1): 512, (2, 0): 1024, (2, 1): 1536,
        (1, 0): 2048, (1, 1): 2304, (3, 0): 2560, (3, 1): 2816}
BLK512 = [(0, 0), (0, 1), (2, 0), (2, 1)]   # N=512 (both q slots)
BLK256 = [(1, 0), (1, 1), (3, 0), (3, 1)]   # N=256 (q slot 1 only)
# PV accumulation order: first/last must be full-width (N=512) blocks.
PV_ORDER = [(0, 0), (1, 0), (1, 1), (3, 0), (3, 1), (0, 1), (2, 0), (2, 1)]


def rne12(x: np.ndarray) -> np.ndarray:
    """Round fp32 to float32r (round-to-nearest-even, drop low 12 bits)."""
    x = np.ascontiguousarray(x, dtype=np.float32)
    u = x.view(np.uint32).astype(np.uint64)
    lsb = (u >> np.uint64(12)) & np.uint64(1)
    u2 = (u + np.uint64(0x7FF) + lsb) >> np.uint64(12) << np.uint64(12)
    return u2.astype(np.uint32).view(np.float32).reshape(x.shape)


def build_program():
    nc = bacc.Bacc("TRN2", target_bir_lowering=False, debug=False,
                   num_devices=(1 if SINGLE else NC))

    # ---- DRAM I/O ----
    d_toksT = nc.dram_tensor("toksT", [V, S], F32R, kind="ExternalInput")
    d_posT = nc.dram_tensor("posT", [E, S], F32, kind="ExternalInput")
    d_maska = nc.dram_tensor("maska", [128, 4, CH], BF16, kind="ExternalInput")
    d_maskb = nc.dram_tensor("maskb", [128, 4 * CH], BF16, kind="ExternalInput")
    d_pboff = nc.dram_tensor("pboff", [1, 1], mybir.dt.uint32, kind="ExternalInput")
    d_reps = (nc.dram_tensor("reps", [1, 1], mybir.dt.uint32,
                             kind="ExternalInput") if HWLOOP > 1 else None)
    d_wqk = nc.dram_tensor("wqk", [L, E, H, 2 * C], F32R, kind="ExternalInput")
    d_wv = nc.dram_tensor("wv", [L, E, H * C], F32R, kind="ExternalInput")
    d_w1 = nc.dram_tensor("w1", [L, E, M], F32R, kind="ExternalInput")
    d_w2 = nc.dram_tensor("w2", [L, M, E], F32R, kind="ExternalInput")
    d_b1 = nc.dram_tensor("b1", [L, M], F32, kind="ExternalInput")
    d_b2 = nc.dram_tensor("b2", [L, E], F32R, kind="ExternalInput")
    d_ln1g = nc.dram_tensor("ln1g", [L, E], F32R, kind="ExternalInput")
    d_ln1b = nc.dram_tensor("ln1b", [L, E], F32R, kind="ExternalInput")
    d_ln2g = nc.dram_tensor("ln2g", [L, E], F32R, kind="ExternalInput")
    d_ln2b = nc.dram_tensor("ln2b", [L, E], F32R, kind="ExternalInput")
    d_lnfg = nc.dram_tensor("lnfg", [E], F32R, kind="ExternalInput")
    d_lnfb = nc.dram_tensor("lnfb", [E], F32R, kind="ExternalInput")
    d_tokw = nc.dram_tensor("tokw", [V, E], F32R, kind="ExternalInput")
    d_uw = nc.dram_tensor("uw", [E, V], F32R, kind="ExternalInput")
    d_ub = nc.dram_tensor("ub", [V], F32R, kind="ExternalInput")
    d_out = nc.dram_tensor("logits", [V, S], F32, kind="ExternalOutput")

    with tile.TileContext(nc) as tc:
        from contextlib import ExitStack
        top = ExitStack()
        pers = top.enter_context(tc.tile_pool(name="pers", bufs=1))
        small = top.enter_context(tc.tile_pool(name="small", bufs=1))
        stage = top.enter_context(tc.tile_pool(name="stage", bufs=2))
        bigpool = top.enter_context(tc.tile_pool(name="bigpool", bufs=1))
        dram = top.enter_context(tc.tile_pool(name="dram", bufs=2, space="DRAM"))
        dram_sh = top.enter_context(tc.tile_pool(name="dram_sh", bufs=2,
                                                 space="DRAM"))

        # ---- persistent SBUF state ----
        x = pers.tile([128, 8, S], F32)        # residual (feature-major)
        inner = pers.tile([128, 8, S], F32)    # x + attn_out (+ mlp out)
        h = pers.tile([128, 8, S], F32R)       # LN output / rounded x
        qsb = pers.tile([128, 8, S], BF16)     # Q (head-pair-major)
        maska = pers.tile([128, 4, CH], BF16)
        maskb = pers.tile([128, 4 * CH], BF16)
        ones_col = pers.tile([128, 1], F32R)   # stats lhsT, carries 1/E
        ones_row = pers.tile([1, 128], F32R)   # broadcast lhsT
        ones_s = pers.tile([1, S], F32R)       # bias outer-product rhs
        g1 = pers.tile([1, E], F32R)
        bb1 = pers.tile([1, E], F32R)
        g2 = pers.tile([1, E], F32R)
        bb2 = pers.tile([1, E], F32R)
        gf = pers.tile([1, E], F32R)
        bf = pers.tile([1, E], F32R)
        b1sb = pers.tile([128, 32], F32)
        b2r = pers.tile([1, E], F32R)
        ubr = pers.tile([1, V], F32R)
        lnst = small.tile([1, 4 * S], F32)     # mu | mu2 | var | rvar
        lnst_r = small.tile([1, 2 * S], F32R)  # rstd | nmu

        nc.vector.memset(ones_col[:].bitcast(F32), 1.0 / E)
        nc.vector.memset(ones_row[:].bitcast(F32), 1.0)
        nc.vector.memset(ones_s[:].bitcast(F32), 1.0)
        nc.sync.dma_start(maska[:], d_maska[:])
        nc.sync.dma_start(maskb[:], d_maskb[:])
        nc.sync.dma_start(gf[:], d_lnfg[None, :])
        nc.sync.dma_start(bf[:], d_lnfb[None, :])
        nc.sync.dma_start(ubr[:], d_ub[None, :])

        # pair-base register: element offset of my pair's even-rank blob
        # in each AllGather output.
        if SINGLE:
            pbi = 0
        else:
            pbreg_t = nc.sync.alloc_register("pbreg")
            nc.sync.reg_load(pbreg_t, d_pboff[0:1, 0:1])
            pbi = nc.sync.snap(pbreg_t, donate=True, min_val=0, max_val=3)

        def layer_norm(src_t, g_row, b_row):
            """src_t: [128,8,S] F32 sbuf -> h (F32R).  h doubles as the
            rounded copy of src that feeds the stats matmuls.
            h = src*outer(g,rstd) + outer(g,-mu*rstd) + outer(b,1)."""
            mu = lnst[:, 0:S]
            mu2 = lnst[:, S:2 * S]
            var = lnst[:, 2 * S:3 * S]
            rvar = lnst[:, 3 * S:4 * S]
            rstd = lnst_r[:, 0:S]
            nmu = lnst_r[:, S:2 * S]
            with tc.tile_pool(name="ps_ln", bufs=2, space="PSUM") as ps_ln:
                for t in range(8):
                    nc.vector.tensor_copy(h[:, t, :], src_t[:, t, :])
                st_mu = ps_ln.tile([1, S], F32, tag="stat", bufs=2)
                st_msq = ps_ln.tile([1, S], F32, tag="stat", bufs=2)
                for t in range(8):
                    nc.tensor.matmul(st_mu[:], ones_col[:], h[:, t, :],
                                     start=(t == 0), stop=(t == 7))
                for t in range(8):
                    sqt = stage.tile([128, S], F32R, tag="sqt", bufs=3)
                    nc.vector.tensor_tensor(sqt[:], src_t[:, t, :],
                                            src_t[:, t, :], AOT.mult)
                    nc.tensor.matmul(st_msq[:], ones_col[:], sqt[:],
                                     start=(t == 0), stop=(t == 7))
                nc.vector.tensor_copy(mu, st_mu[:])
                nc.vector.tensor_tensor(mu2, mu, mu, AOT.mult)
                nc.vector.tensor_tensor(var, st_msq[:], mu2, AOT.subtract)
                nc.vector.tensor_scalar_add(var, var, EPS)
                nc.vector.reciprocal(rvar, var)
                with nc.allow_low_precision(reason="fp32r rstd for bcast"):
                    nc.scalar.sqrt(rstd, rvar)
                nc.vector.tensor_tensor(nmu, mu, rstd, AOT.mult)
                with nc.allow_low_precision(reason="fp32r nmu for bcast"):
                    nc.vector.tensor_scalar_mul(nmu, nmu, -1.0)
                for t in range(8):
                    bc1 = ps_ln.tile([128, S], F32, tag="bc", bufs=4)
                    bc2 = ps_ln.tile([128, S], F32, tag="bc", bufs=4)
                    gsl = g_row[:, t * 128:(t + 1) * 128]
                    bsl = b_row[:, t * 128:(t + 1) * 128]
                    nc.tensor.matmul(bc1[:], gsl, rstd, start=True, stop=True)
                    nc.tensor.matmul(bc2[:], gsl, nmu, start=True, stop=False)
                    nc.tensor.matmul(bc2[:], bsl, ones_s[:], start=False,
                                     stop=True)
                    tmp = stage.tile([128, S], F32, tag="lntmp", bufs=3)
                    nc.vector.tensor_tensor(tmp[:], src_t[:, t, :], bc1[:],
                                            AOT.mult)
                    nc.vector.tensor_tensor(h[:, t, :], tmp[:], bc2[:], AOT.add)

        def body():
            # ---------------- embedding ----------------
            with tc.tile_pool(name="emb", bufs=1) as emb, \
                 tc.tile_pool(name="wemb", bufs=2) as wemb, \
                 tc.tile_pool(name="ps_emb", bufs=4, space="PSUM") as ps_emb:
                toksr = emb.tile([128, 8, S], F32R)
                for v in range(8):
                    nc.sync.dma_start(toksr[:, v, :],
                                      d_toksT[v * 128:(v + 1) * 128, :])
                for eg in range(4):
                    psl = [ps_emb.tile([128, S], F32, tag="embps",
                                       name=f"embps{_i}") for _i in range(2)]
                    for vh in range(4):
                        tw = wemb.tile([128, 2, 256], F32R, tag="twt")
                        for v2 in range(2):
                            nc.sync.dma_start(
                                tw[:, v2, :],
                                d_tokw[(vh * 2 + v2) * 128:
                                       (vh * 2 + v2 + 1) * 128,
                                       eg * 256:(eg + 1) * 256])
                        for v2 in range(2):
                            v = vh * 2 + v2
                            for eo2 in range(2):
                                nc.tensor.matmul(
                                    psl[eo2][:],
                                    tw[:, v2, eo2 * 128:(eo2 + 1) * 128],
                                    toksr[:, v, :],
                                    start=(v == 0), stop=(v == 7))
                    for eo2 in range(2):
                        nc.scalar.copy(x[:, eg * 2 + eo2, :], psl[eo2][:])
            with tc.tile_pool(name="embp", bufs=1) as embp:
                possb = embp.tile([128, 8, S], F32)
                for e in range(8):
                    nc.sync.dma_start(possb[:, e, :],
                                      d_posT[e * 128:(e + 1) * 128, :])
                    nc.vector.tensor_tensor(x[:, e, :], x[:, e, :],
                                            possb[:, e, :], AOT.add)

            # ---------------- layers ----------------
            for l in range(N_LAYERS):
                nc.sync.dma_start(g1[:], d_ln1g[l][None, :])
                nc.sync.dma_start(bb1[:], d_ln1b[l][None, :])
                nc.sync.dma_start(g2[:], d_ln2g[l][None, :])
                nc.sync.dma_start(bb2[:], d_ln2b[l][None, :])
                nc.sync.dma_start(b1sb[:],
                                  d_b1[l].rearrange("(o p) -> p o", p=128))
                nc.sync.dma_start(b2r[:], d_b2[l][None, :])

                agk_in = [dram.tile([QBLOB], BF16, tag=f"agkin{_i}",
                                    name=f"agkin{_i}") for _i in range(2)]
                agv_in = dram.tile([HBLOB], BF16, tag="agvin")
                agk_out = [dram_sh.tile(
                    [NC * QBLOB], BF16, tag=f"agkout{_i}", name=f"agkout{_i}",
                    addr_space=("Local" if FAKE_AG else "Shared"))
                    for _i in range(2)]
                agv_out = dram_sh.tile(
                    [NC * HBLOB], BF16, tag="agvout",
                    addr_space=("Local" if FAKE_AG else "Shared"))
                # quarter blob hg: K rows [hg*512 : (hg+1)*512] -> [512, 512]
                agi_k = [agk_in[_i][:].rearrange("(r c) -> r c", c=S)
                         for _i in range(2)]

                # ---- LN1 ----
                layer_norm(x, g1, bb1)

                # ---- V projection first (token-major); its AllGather
                # then overlaps the Q/K projection ----
                with tc.tile_pool(name="wqkv", bufs=2) as wqkv, \
                     tc.tile_pool(name="ps_qkv", bufs=8, space="PSUM") as ps_qkv:
                    v_ps = [[ps_qkv.tile([128, 512], F32, tag="qkps", bufs=8,
                                         name=f"vps{_i}_{_j}")
                             for _j in range(2)] for _i in range(4)]
                    for e in range(8):
                        wv = wqkv.tile([128, 1024], F32R, tag="wv", bufs=3)
                        nc.sync.dma_start(wv[:],
                                          d_wv[l, e * 128:(e + 1) * 128, :])
                        for tt in range(4):
                            for hf in range(2):
                                nc.tensor.matmul(
                                    v_ps[tt][hf][:],
                                    h[:, e, tt * 128:(tt + 1) * 128],
                                    wv[:, hf * 512:(hf + 1) * 512],
                                    start=(e == 0), stop=(e == 7))
                    for tt in range(4):
                        vst = stage.tile([128, 1024], BF16, tag="vst", bufs=2)
                        nc.scalar.copy(vst[:, 0:512], v_ps[tt][0][:])
                        nc.scalar.copy(vst[:, 512:1024], v_ps[tt][1][:])
                        nc.sync.dma_start(
                            agv_in[tt * 128 * 1024:(tt + 1) * 128 * 1024]
                            .rearrange("(r c) -> r c", c=1024),
                            vst[:])
                    if FAKE_AG:
                        nc.sync.dma_start(agv_out[0:HBLOB], agv_in[:])
                        nc.sync.dma_start(agv_out[HBLOB:2 * HBLOB], agv_in[:])
                    else:
                        nc.gpsimd.collective_compute(
                            "AllGather", AOT.bypass,
                            replica_groups=[list(range(NC))],
                            ins=[agv_in.opt()], outs=[agv_out.opt()])
                    # ---- Q,K projection (K straight to DRAM) ----
                    for hg in range(2):
                        qk_ps = [ps_qkv.tile([128, S], F32, tag="qkps", bufs=8,
                                             name=f"qkps{_i}")
                                 for _i in range(8)]
                        for e in range(8):
                            wq = wqkv.tile([128, 8, 128], F32R, tag="wqk",
                                           bufs=3)
                            nc.sync.dma_start(
                                wq[:], d_wqk[l, e * 128:(e + 1) * 128,
                                             hg * 8:(hg + 1) * 8, :])
                            for hh in range(8):
                                nc.tensor.matmul(qk_ps[hh][:], wq[:, hh, :],
                                                 h[:, e, :],
                                                 start=(e == 0), stop=(e == 7))
                        for hh in range(8):
                            ha = hg * 8 + hh
                            hp, par = ha // 2, ha % 2
                            nc.scalar.copy(qsb[par * 64:par * 64 + 64, hp, :],
                                           qk_ps[hh][0:64, :])
                            kst = stage.tile([64, S], BF16, tag="kst", bufs=3)
                            nc.scalar.copy(kst[:], qk_ps[hh][64:128, :])
                            row = (hp % 4) * 128 + par * 64
                            nc.sync.dma_start(agi_k[hg][row:row + 64, :],
                                              kst[:])
                        # K exchange for this head group (overlaps the next
                        # projection pass / early attention)
                        if FAKE_AG:
                            nc.sync.dma_start(agk_out[hg][0:QBLOB],
                                              agk_in[hg][:])
                            nc.sync.dma_start(agk_out[hg][QBLOB:2 * QBLOB],
                                              agk_in[hg][:])
                        else:
                            nc.gpsimd.collective_compute(
                                "AllGather", AOT.bypass,
                                replica_groups=[list(range(NC))],
                                ins=[agk_in[hg].opt()],
                                outs=[agk_out[hg].opt()])

                # gathered V (token-major, with interleaved ones column)
                vgr = bigpool.tile([128, 8, H, C + 1], BF16, tag="vgrm",
                                   bufs=1)
                for kt in range(8):
                    off = (pbi * (2 * HBLOB) + (0 if kt < 4 else HBLOB)
                           + (kt % 4) * 128 * 1024)
                    vsrc = agv_out[bass.ds(off, 128 * 1024)].rearrange(
                        "(r hh cc) -> r hh cc", hh=H, cc=C)
                    nc.sync.dma_start(vgr[:, kt, :, 0:C], vsrc)
                nc.vector.memset(vgr[:, :, :, C:C + 1], 1.0)

                # ---- attention ----
                with tc.tile_pool(name="attnp", bufs=1) as attnp, \
                     tc.tile_pool(name="ps_s", bufs=4, space="PSUM") as ps_s, \
                     tc.tile_pool(name="ps_y", bufs=2, space="PSUM") as ps_y, \
                     tc.tile_pool(name="ps_by", bufs=2, space="PSUM") as ps_by:
                    for hp in range(8):
                        kr = attnp.tile([128, 1024], BF16, tag="kr", bufs=3)
                        hg = hp // 4
                        base = pbi * (2 * QBLOB) + (hp % 4) * 128 * S
                        nc.sync.dma_start(
                            kr[:, 0:512],
                            agk_out[hg][bass.ds(base, 128 * S)]
                            .rearrange("(r c) -> r c", c=S))
                        nc.sync.dma_start(
                            kr[:, 512:1024],
                            agk_out[hg][bass.ds(base + QBLOB, 128 * S)]
                            .rearrange("(r c) -> r c", c=S))
                        for head in range(2):
                            ha = hp * 2 + head
                            qb = head * 64
                            P = attnp.tile([128, 3072], BF16, tag="P", bufs=2)
                            for (g, kt) in BLK512:
                                ktg = g * 2 + kt
                                sps = ps_s.tile([128, 512], F32, tag="S",
                                                bufs=4)
                                nc.tensor.matmul(
                                    sps[:],
                                    kr[qb:qb + 64,
                                       ktg * 128:(ktg + 1) * 128],
                                    qsb[qb:qb + 64, hp, :],
                                    start=True, stop=True)
                                pc = PCOL[(g, kt)]
                                nc.scalar.activation(P[:, pc:pc + 512], sps[:],
                                                     AFT.Exp, scale=SCALE)
                            for (g, kt) in BLK256:
                                ktg = g * 2 + kt
                                sps = ps_s.tile([128, 256], F32, tag="S",
                                                bufs=4)
                                nc.tensor.matmul(
                                    sps[:],
                                    kr[qb:qb + 64,
                                       ktg * 128:(ktg + 1) * 128],
                                    qsb[qb:qb + 64, hp, 256:512],
                                    start=True, stop=True)
                                pc = PCOL[(g, kt)]
                                nc.scalar.activation(P[:, pc:pc + 256], sps[:],
                                                     AFT.Exp, scale=SCALE)
                            # causal mask (0/1 multiplicative)
                            ap1 = P[:, 0:2048].rearrange(
                                "p (b q) -> p b q", q=512)[:, :, 0:CH]
                            nc.vector.tensor_tensor(ap1, ap1, maska[:],
                                                    AOT.mult)
                            ap2 = P[:, 2048:3072]
                            nc.vector.tensor_tensor(ap2, ap2, maskb[:],
                                                    AOT.mult)
                            # PV (+ denominator via ones column)
                            yps = ps_y.tile([128, 512], F32, tag="y", bufs=2)
                            for i, (g, kt) in enumerate(PV_ORDER):
                                ktg = g * 2 + kt
                                pc = PCOL[(g, kt)]
                                n = 512 if (g, kt) in BLK512 else 256
                                qoff = 0 if n == 512 else 256
                                nc.tensor.matmul(
                                    yps[0:65, qoff:qoff + n],
                                    vgr[:, ktg, ha, :],
                                    P[:, pc:pc + n],
                                    start=(i == 0),
                                    stop=(i == len(PV_ORDER) - 1))
                            rd = small.tile([1, S], F32R, tag="rd", bufs=2)
                            with nc.allow_low_precision(reason="1/d bcast"):
                                nc.vector.reciprocal(rd[:], yps[64:65, :])
                            bcd = ps_by.tile([64, S], F32, tag="bcd", bufs=2)
                            nc.tensor.matmul(bcd[:], ones_row[:, 0:64], rd[:],
                                             start=True, stop=True)
                            et, ebase = ha // 2, (ha % 2) * 64
                            ysb = stage.tile([128, S], F32, tag="ysb", bufs=2)
                            ysl = ysb[ebase:ebase + 64, :]
                            nc.scalar.copy(ysl, yps[0:64, :])
                            nc.vector.tensor_tensor(ysl, ysl, bcd[:], AOT.mult)
                            # inner = x + y
                            nc.vector.tensor_tensor(
                                inner[ebase:ebase + 64, et, :],
                                x[ebase:ebase + 64, et, :], ysl, AOT.add)

                # ---- LN2 + MLP (mlp out accumulates into `inner`) ----
                layer_norm(inner, g2, bb2)
                with tc.tile_pool(name="wmlp", bufs=1) as wmlp, \
                     tc.tile_pool(name="ps_m", bufs=4, space="PSUM") as ps_m, \
                     tc.tile_pool(name="ps_o", bufs=4, space="PSUM") as ps_o:
                    for half in range(2):
                        m_sb = bigpool.tile([128, 16, 512], F32R, tag="vgrm",
                                            bufs=1)
                        for og4 in range(4):
                            og = half * 4 + og4
                            mps_l = [ps_m.tile([128, S], F32, tag="mps",
                                               bufs=4, name=f"mps{_i}")
                                     for _i in range(4)]
                            for eh in range(2):
                                w1h = wmlp.tile([128, 4, 512], F32R,
                                                tag="w1h", bufs=2)
                                for e4 in range(4):
                                    e = eh * 4 + e4
                                    nc.sync.dma_start(
                                        w1h[:, e4, :],
                                        d_w1[l, e * 128:(e + 1) * 128,
                                             og * 512:(og + 1) * 512])
                                for ob in range(4):
                                    for e4 in range(4):
                                        e = eh * 4 + e4
                                        nc.tensor.matmul(
                                            mps_l[ob][:],
                                            w1h[:, e4,
                                                ob * 128:(ob + 1) * 128],
                                            h[:, e, :],
                                            start=(e == 0), stop=(e == 7))
                            for ob in range(4):
                                mtl = og4 * 4 + ob
                                mt_abs = half * 16 + mtl
                                nc.scalar.activation(
                                    m_sb[:, mtl, :], mps_l[ob][:], AFT.Relu,
                                    bias=b1sb[:, mt_abs:mt_abs + 1], scale=1.0)
                        for eoh in range(2):
                            ops = [ps_o.tile([128, S], F32, tag="ops", bufs=4,
                                             name=f"ops{_i}")
                                   for _i in range(4)]
                            if half == 0:
                                for eo4 in range(4):
                                    eo = eoh * 4 + eo4
                                    nc.tensor.matmul(
                                        ops[eo4][:],
                                        b2r[:, eo * 128:(eo + 1) * 128],
                                        ones_s[:], start=True, stop=False)
                            for mt in range(16):
                                mt_abs = half * 16 + mt
                                w2t = wmlp.tile([128, 512], F32R, tag="w2",
                                                bufs=4)
                                nc.sync.dma_start(
                                    w2t[:],
                                    d_w2[l, mt_abs * 128:(mt_abs + 1) * 128,
                                         eoh * 512:(eoh + 1) * 512])
                                for eo4 in range(4):
                                    nc.tensor.matmul(
                                        ops[eo4][:],
                                        w2t[:, eo4 * 128:(eo4 + 1) * 128],
                                        m_sb[:, mt, :],
                                        start=(half == 1 and mt == 0),
                                        stop=(mt == 15))
                            for eo4 in range(4):
                                eo = eoh * 4 + eo4
                                nc.vector.tensor_tensor(inner[:, eo, :],
                                                        inner[:, eo, :],
                                                        ops[eo4][:], AOT.add)
                # x = x + inner  (inner holds x + attn + mlp + b2)
                for e in range(8):
                    nc.vector.tensor_tensor(x[:, e, :], x[:, e, :],
                                            inner[:, e, :], AOT.add)

            # ---------------- final LN + unembed ----------------
            if DEBUG_X:
                for e in range(8):
                    xs = stage.tile([128, S], F32, tag="lntmp", bufs=3)
                    nc.vector.tensor_copy(xs[:], x[:, e, :])
                    nc.sync.dma_start(d_out[e * 128:(e + 1) * 128, :], xs[:])
            else:
                layer_norm(x, gf, bf)
                with tc.tile_pool(name="wu", bufs=2) as wu, \
                     tc.tile_pool(name="ps_u", bufs=5, space="PSUM") as ps_u:
                    for vg in range(2):
                        upl = [ps_u.tile([128, S], F32, tag="ups", bufs=5,
                                         name=f"ups{_i}") for _i in range(4)]
                        for vo4 in range(4):
                            vo = vg * 4 + vo4
                            nc.tensor.matmul(upl[vo4][:],
                                             ubr[:, vo * 128:(vo + 1) * 128],
                                             ones_s[:], start=True, stop=False)
                        for eh in range(2):
                            uwh = wu.tile([128, 4, 512], F32R, tag="uwh",
                                          bufs=2)
                            for e4 in range(4):
                                e = eh * 4 + e4
                                nc.sync.dma_start(
                                    uwh[:, e4, :],
                                    d_uw[e * 128:(e + 1) * 128,
                                         vg * 512:(vg + 1) * 512])
                            for vo4 in range(4):
                                for e4 in range(4):
                                    e = eh * 4 + e4
                                    nc.tensor.matmul(
                                        upl[vo4][:],
                                        uwh[:, e4, vo4 * 128:(vo4 + 1) * 128],
                                        h[:, e, :], start=False,
                                        stop=(e == 7))
                        for vo4 in range(4):
                            vo = vg * 4 + vo4
                            lst = stage.tile([128, S], F32, tag="lntmp",
                                             bufs=3)
                            nc.scalar.copy(lst[:], upl[vo4][:])
                            nc.sync.dma_start(
                                d_out[vo * 128:(vo + 1) * 128, :], lst[:])

        if HWLOOP > 1:
            rtmp = nc.alloc_registers("reps_reg", mybir.ALL_ENGINES)
            nc.regs_load(rtmp, d_reps[0:1, 0:1])
            rv = nc.snap(rtmp, donate=True, min_val=1, max_val=1 << 20)
            with tc.For_i(0, rv, 1):
                body()
        else:
            body()

        top.close()

    nc.compile()
    return nc


def core_token_idx(c: int) -> np.ndarray:
    p = c % 2
    c0, c1 = QCH[p]
    return np.concatenate([np.arange(c0 * CH, (c0 + 1) * CH),
                           np.arange(c1 * CH, (c1 + 1) * CH)])


def build_masks(parity: int):
    """Multiplicative 0/1 masks in the P-tile layout (bf16)."""
    import ml_dtypes
    k_idx = np.arange(128)
    q_idx = np.arange(CH)

    def blk(g, kt, qs):
        lk = GORDER[g]
        lq = QCH[parity][qs]
        kk = lk * CH + kt * 128 + k_idx[:, None]
        qq = lq * CH + q_idx[None, :]
        return (kk <= qq).astype(np.float32)

    maska = np.stack([blk(0, 0, 0), blk(0, 1, 0), blk(2, 0, 0), blk(2, 1, 0)],
                     axis=1).astype(ml_dtypes.bfloat16)
    maskb = np.concatenate(
        [blk(1, 0, 1), blk(1, 1, 1), blk(3, 0, 1), blk(3, 1, 1)],
        axis=1).astype(ml_dtypes.bfloat16)
    return np.ascontiguousarray(maska), np.ascontiguousarray(maskb)


_NC_CACHE = None


def prepare_in_maps(inputs):
    toks = np.asarray(inputs["toks"], np.float32)
    pos_W = np.asarray(inputs["pos_W"], np.float32)
    attn_W = np.asarray(inputs["attn_W"], np.float32)

    aw = attn_W.reshape(L, E, H, 3 * C)
    shared = {
        "wqk": rne12(np.ascontiguousarray(aw[:, :, :, 0:2 * C])),
        "wv": rne12(np.ascontiguousarray(aw[:, :, :, 2 * C:]).reshape(L, E, H * C)),
        "w1": rne12(np.asarray(inputs["mlp_W1"], np.float32)),
        "w2": rne12(np.asarray(inputs["mlp_W2"], np.float32)),
        "b1": np.ascontiguousarray(inputs["mlp_b1"], np.float32),
        "b2": rne12(np.asarray(inputs["mlp_b2"], np.float32)),
        "ln1g": rne12(np.asarray(inputs["ln1_g"], np.float32)),
        "ln1b": rne12(np.asarray(inputs["ln1_b"], np.float32)),
        "ln2g": rne12(np.asarray(inputs["ln2_g"], np.float32)),
        "ln2b": rne12(np.asarray(inputs["ln2_b"], np.float32)),
        "lnfg": rne12(np.asarray(inputs["lnf_g"], np.float32)),
        "lnfb": rne12(np.asarray(inputs["lnf_b"], np.float32)),
        "tokw": rne12(np.asarray(inputs["tok_W"], np.float32)),
        "uw": rne12(np.asarray(inputs["unembed_W"], np.float32)),
        "ub": rne12(np.asarray(inputs["unembed_b"], np.float32)),
    }
    in_maps = []
    for c in range(NC):
        b, p = c // 2, c % 2
        idx = core_token_idx(c)
        ma, mb = build_masks(p)
        m = dict(shared)
        m["toksT"] = rne12(np.ascontiguousarray(toks[b, idx, :].T))
        m["posT"] = np.ascontiguousarray(pos_W[idx, :].T)
        m["maska"] = ma
        m["maskb"] = mb
        m["pboff"] = np.array([[c // 2]], dtype=np.uint32)
        if HWLOOP > 1:
            m["reps"] = np.array(
                [[int(os.environ.get("KERNEL_REPS", "1"))]], dtype=np.uint32)
        in_maps.append(m)
    return in_maps


def kernel(**inputs) -> np.ndarray:
    global _NC_CACHE
    if _NC_CACHE is None:
        _NC_CACHE = build_program()
    nc = _NC_CACHE
    in_maps = prepare_in_maps(inputs)

    r = run_bass_kernel_spmd(nc, in_maps, core_ids=list(range(NC)))

    out = np.empty((B, T, V), np.float32)
    for c in range(NC):
        b = c // 2
        idx = core_token_idx(c)
        out[b, idx, :] = r.results[c]["logits"].T
    return out


if __name__ == "__main__":
    print("building program...")
    nc0 = build_program()
    print("built ok")
